# revision 54
# baseline (speedup 1.0000x reference)
"""Trainium2 Bass kernel for causal linear-complexity multi-head attention.

Reference computation (per batch n):
    q = softmax((query @ Wq.T) per-head, axis=Dh)
    k = softmax((key  @ Wk.T) per-head, axis=Dh)
    v = key @ Wv.T
    S[t] = sum_{s<=t} k_s^T v_s          (per-head Dh x Dh running state)
    out[t] = q_t @ S[t]

Sharding: 8 cores = 2 batches x 4 head-groups (4 heads of 64 dims each per
core).  Each core gets host-transposed inputs (d on rows); per-core output
is the natural-layout (L x 256) slice; the host concatenates.

Device algorithm: chunked linear attention, chunk C=256.

Projections run as fp8e4m3 DoubleRow matmuls (2 contraction planes per
matmul, 0.5 cycles/row) with host-prepared hi/lo splits:
    x = xh + xl,  W = Wh + Wl   (fp8 round, residual re-rounded)
    q,v: x@W ~= xh@Wh + xl@Wh + xh@Wl   (lo*lo dropped, ~1e-3 rel err)
    k:   x@W ~= xh@Wh + xh@Wl           (k tolerates a plain-fp8 x; the
                                         cumulative state averages it out)
W is pre-scaled by 64 on host to center it in fp8 normal range; undone by
the exp activation scale (q/k paths) and a 64-valued k-denominator ones
vector (v path, scale rides the existing per-head reciprocal).

The attention block runs in bf16 and produces the output in NATURAL layout:
    scores   pat[s, t] = ek^T eq        (per head, T layout, masked on DVE)
    output   out[t, j] = at^T v + eq^T S
so the per-(head, t-tile) output matmuls are 64-wide (24 x 64 cycles versus
12 x 256 in T layout), and the q-softmax denominators reduce to a [128, 8]
reciprocal applied via a stride-0 broadcast multiply.  The fp32 state S
accumulates on DVE; a bf16 shadow copy (gpsimd) feeds the apply matmuls.
"""

import threading
from contextlib import ExitStack

import numpy as np

import concourse.bass as bass
import concourse.mybir as mybir
import concourse.tile as tile
from concourse import bacc
from concourse.bass_utils import run_bass_kernel_spmd

P = 128          # SBUF partitions
D = 1024         # model dim (contraction)
J = 256          # per-core output columns (4 heads x 64)
L = 2048         # sequence length
C = 256          # chunk size
NCH = L // C     # chunks
DH = 64          # per-head dim
KB = 4           # 256-deep contraction blocks
N_CORES = 8
WSCALE = 64.0    # host pre-scale on W for fp8 range

F32 = mybir.dt.float32
F32R = mybir.dt.float32r
BF16 = mybir.dt.bfloat16
FP8 = mybir.dt.float8e4
EXP = mybir.ActivationFunctionType.Exp
COPY = mybir.ActivationFunctionType.Copy
DR = mybir.MatmulPerfMode.DoubleRow

# fp8 DR term list per projection: (w hi/lo, x hi/lo) — lo*lo dropped.
# x-hi terms first so chunk-0 can start before the x-lo DMA lands.
TERMS = ((0, 0), (1, 0), (0, 1))
KTERMS = ((0, 0), (1, 0))


def _build_nc():
    nc = bacc.Bacc(trn_type="TRN2", target_bir_lowering=False, num_devices=N_CORES)

    # x: [p, chunk, hi/lo, kb, plane, t]  with d = kb*256 + plane*128 + p
    xq = nc.dram_tensor("xq", [P, NCH, 2, KB, 2, C], FP8, kind="ExternalInput").ap()
    xk = nc.dram_tensor("xk", [P, NCH, 2, KB, 2, C], FP8, kind="ExternalInput").ap()
    # w: [p, proj(q,k,v), hi/lo, kb, plane, j]
    wa = nc.dram_tensor("wa", [P, 3, 2, KB, 2, J], FP8, kind="ExternalInput").ap()
    mask = nc.dram_tensor("mask", [P, 2, C], F32R, kind="ExternalInput").ap()
    eones1 = nc.dram_tensor("eones1", [P, 2], BF16, kind="ExternalInput").ap()
    eones64 = nc.dram_tensor("eones64", [P, 2], BF16, kind="ExternalInput").ap()
    ident = nc.dram_tensor("ident", [P, P], BF16, kind="ExternalInput").ap()
    out_d = nc.dram_tensor("out_nat", [L, J], F32, kind="ExternalOutput").ap()

    # natural-layout store: row = c*256 + tt*128 + p
    out_r = out_d.rearrange("(c tt p) j -> p c tt j", p=P, tt=2)

    with tile.TileContext(nc) as tc, ExitStack() as ctx:
        ctx.enter_context(
            nc.allow_low_precision(reason="fp8/bf16 compensated pipeline")
        )
        cpool = ctx.enter_context(tc.tile_pool(name="consts", bufs=1))
        xpool = ctx.enter_context(tc.tile_pool(name="xin", bufs=6))
        spool = ctx.enter_context(tc.tile_pool(name="sb", bufs=12))
        ppool = ctx.enter_context(tc.tile_pool(name="pp", bufs=3, space="PSUM"))
        patp = ctx.enter_context(tc.tile_pool(name="pa", bufs=3, space="PSUM"))
        potp = ctx.enter_context(tc.tile_pool(name="po", bufs=1, space="PSUM"))
        pnorm = ctx.enter_context(tc.tile_pool(name="pn", bufs=1, space="PSUM"))

        # ---- constants / chunk-0 inputs ----
        # DMA issue order = SP-ring FIFO order: interleave weights with
        # chunk-0 activations per projection, quarter-split for the first
        # pair, so the first matmuls start as early as possible.
        wa_sb = cpool.tile([P, 3, 2, KB, 2, J], FP8, tag="wa_sb")
        xq_t0 = xpool.tile([P, 2, KB, 2, C], FP8, tag="xq_t", name="xq_t0")
        xk_t0 = xpool.tile([P, 2, KB, 2, C], FP8, tag="xk_t", name="xk_t0")
        nc.sync.dma_start(wa_sb[:, 0, 0, 0:2], wa[:, 0, 0, 0:2])
        nc.sync.dma_start(xq_t0[:, 0, 0:2], xq[:, 0, 0, 0:2])
        nc.sync.dma_start(wa_sb[:, 0, 0, 2:4], wa[:, 0, 0, 2:4])
        nc.sync.dma_start(xq_t0[:, 0, 2:4], xq[:, 0, 0, 2:4])
        nc.sync.dma_start(wa_sb[:, 0, 1], wa[:, 0, 1])
        nc.sync.dma_start(xq_t0[:, 1], xq[:, 0, 1])
        nc.sync.dma_start(wa_sb[:, 1, 0], wa[:, 1, 0])
        nc.sync.dma_start(xk_t0[:, 0], xk[:, 0, 0])
        nc.sync.dma_start(wa_sb[:, 1, 1], wa[:, 1, 1])
        nc.sync.dma_start(xk_t0[:, 1], xk[:, 0, 1])
        nc.sync.dma_start(wa_sb[:, 2, 0], wa[:, 2, 0])
        nc.sync.dma_start(wa_sb[:, 2, 1], wa[:, 2, 1])
        eones1_sb = cpool.tile([P, 2], BF16, tag="eones1_sb")
        nc.sync.dma_start(eones1_sb[:], eones1[:])
        eones64_sb = cpool.tile([P, 2], BF16, tag="eones64_sb")
        nc.sync.dma_start(eones64_sb[:], eones64[:])
        ident_sb = cpool.tile([P, P], BF16, tag="ident_sb")
        nc.sync.dma_start(ident_sb[:], ident[:])
        mask_sb = cpool.tile([P, 2, C], F32R, tag="mask_sb")
        nc.sync.dma_start(mask_sb[:], mask[:])

        # chunk-1 loads issued right after the constants: together with the
        # per-chunk issue below, x loads stay two chunks ahead of compute
        xq_t1 = xpool.tile([P, 2, KB, 2, C], FP8, tag="xq_t", name="xq_t1")
        nc.sync.dma_start(xq_t1[:], xq[:, 1])
        xk_t1 = xpool.tile([P, 2, KB, 2, C], FP8, tag="xk_t", name="xk_t1")
        nc.sync.dma_start(xk_t1[:], xk[:, 1])

        S_sb = cpool.tile([P, 2, DH], F32, tag="S_sb")
        S16 = cpool.tile([P, 2, DH], BF16, tag="S16")
        # ping-pong masked-score tiles [parity][jt][half]; [:, 1, 0:128] is
        # always zero (s-tile 1 never attends to the first t-half)
        at_mt = {}
        for par in range(2):
            for jt in range(2):
                for half in range(2):
                    t = cpool.tile([P, 2, C], BF16, tag=f"at_m{par}{jt}{half}",
                                   name=f"at_m{par}{jt}{half}")
                    nc.vector.memset(t[:, 1, 0:P], 0.0)
                    at_mt[(par, jt, half)] = t

        def dr_proj_T(pe_t, proj, x_t, terms=TERMS):
            """q/k projection in transposed layout: out[j, t] per jt.
            Term-major so chunk-0 starts as each hi/lo DMA lands."""
            for jt in range(2):
                n = 0
                for whl, xhl in terms:
                    for kb in range(KB):
                        nc.tensor.matmul(
                            pe_t[:, jt, :],
                            wa_sb[:, proj, whl, kb, :, jt * P:(jt + 1) * P],
                            x_t[:, xhl, kb, :, :],
                            start=(n == 0),
                            stop=(n == len(terms) * KB - 1),
                            perf_mode=DR,
                        )
                        n += 1

        xtiles = {0: (xq_t0, xk_t0), 1: (xq_t1, xk_t1)}
        for c in range(NCH):
            last = c == NCH - 1

            if c + 2 < NCH:
                xq_n = xpool.tile([P, 2, KB, 2, C], FP8, tag="xq_t", name="xq_n")
                nc.sync.dma_start(xq_n[:], xq[:, c + 2])
                xk_n = xpool.tile([P, 2, KB, 2, C], FP8, tag="xk_t", name="xk_n")
                nc.sync.dma_start(xk_n[:], xk[:, c + 2])
                xtiles[c + 2] = (xq_n, xk_n)
            xq_t, xk_t = xtiles.pop(c)

            # ---- q/k projections (transposed layout) + exp ----
            eq_e = spool.tile([P, 2, C], BF16, tag="eq_e")
            ek_e = spool.tile([P, 2, C], BF16, tag="ek_e")
            pq_t = ppool.tile([P, 2, C], F32, tag="pp")
            dr_proj_T(pq_t, 0, xq_t)
            nc.scalar.activation(eq_e[:], pq_t[:], EXP, scale=1.0 / WSCALE)
            pk_t = ppool.tile([P, 2, C], F32, tag="pp")
            dr_proj_T(pk_t, 1, xk_t, terms=KTERMS)
            nc.scalar.activation(ek_e[:], pk_t[:], EXP, scale=1.0 / WSCALE)

            # ---- q denominators (natural layout, tiny) ----
            # pdq[t, tt, jt, h2] = dq[head(jt,h2), tt*128+t]
            pdq = pnorm.tile([P, 2, 2, 2], F32, tag="pn", name="pdq")
            for tt in range(2):
                for jt in range(2):
                    nc.tensor.matmul(
                        pdq[:, tt, jt, :],
                        eq_e[:, jt, tt * P:(tt + 1) * P],
                        eones1_sb[:],
                        start=True,
                        stop=True,
                    )
            rq4 = spool.tile([P, 2, 2, 2], F32, tag="rq4")
            nc.vector.reciprocal(rq4[:], pdq[:])

            # ---- v projection (natural layout) ----
            pv_t = ppool.tile([P, 2, J], F32, tag="pp")
            for st in range(2):
                n = 0
                for whl, xhl in TERMS:
                    for kb in range(KB):
                        nc.tensor.matmul(
                            pv_t[:, st, :],
                            xk_t[:, xhl, kb, :, st * P:(st + 1) * P],
                            wa_sb[:, 2, whl, kb, :, :],
                            start=(n == 0),
                            stop=(n == 3 * KB - 1),
                            perf_mode=DR,
                        )
                        n += 1

            # ---- k denominators (64x, natural layout) -> v scale ----
            # pdk[s, jt, th, h2] = 64 * dk[head(jt,h2), th*128+s]
            pdk = pnorm.tile([P, 2, 2, 2], F32, tag="pn", name="pdk")
            for jt in range(2):
                for th in range(2):
                    nc.tensor.matmul(
                        pdk[:, jt, th, :],
                        ek_e[:, jt, th * P:(th + 1) * P],
                        eones64_sb[:],
                        start=True,
                        stop=True,
                    )
            rk = spool.tile([P, 2, 2, 2], F32, tag="rk")
            nc.vector.reciprocal(rk[:], pdk[:])
            v_sb = spool.tile([P, 2, J], BF16, tag="v_sb")
            # st0 scale: one DVE broadcast multiply right after rk (halves
            # the ACT chain that paces the output matmuls); st1 stays on ACT
            rk0_b = rk[:, :, 0, :, None].broadcast_to([P, 2, 2, DH])
            nc.vector.tensor_mul(
                v_sb[:, 0, :].rearrange("p (jt h2 d) -> p jt h2 d",
                                        jt=2, h2=2),
                pv_t[:, 0, :].rearrange("p (jt h2 d) -> p jt h2 d",
                                        jt=2, h2=2),
                rk0_b,
            )
            rk1_b = rk[:, :, 1, :, None].broadcast_to([P, 2, 2, DH])
            nc.vector.tensor_mul(
                v_sb[:, 1, :].rearrange("p (jt h2 d) -> p jt h2 d",
                                        jt=2, h2=2),
                pv_t[:, 1, :].rearrange("p (jt h2 d) -> p jt h2 d",
                                        jt=2, h2=2),
                rk1_b,
            )


            # ---- transpose ek to natural layout (not needed after the
            # last state update) ----
            if not last:
                ekn_sb = spool.tile([P, 2, J], BF16, tag="ekn_sb")
                for jt in range(2):
                    ptr = patp.tile([P, 2, P], BF16, tag="pa")
                    for st in range(2):
                        nc.tensor.transpose(
                            ptr[:, st, :], ek_e[:, jt, st * P:(st + 1) * P],
                            ident_sb[:]
                        )
                    nc.scalar.activation(
                        ekn_sb[:, :, jt * P:(jt + 1) * P], ptr[:], COPY
                    )

            # ---- attention scores (masked on DVE as each tile finishes) ----
            for jt in range(2):
                for half in range(2):
                    rows = slice(64 * half, 64 * half + 64)
                    pat = patp.tile([P, 2, C], F32, tag="pa",
                                    name=f"pat{jt}{half}")
                    for st in range(2):
                        nc.tensor.matmul(
                            pat[:, st, :],
                            ek_e[rows, jt, st * P:(st + 1) * P],
                            eq_e[rows, jt, :],
                            start=True,
                            stop=True,
                        )
                    at_m = at_mt[(c % 2, jt, half)]
                    nc.vector.tensor_mul(
                        at_m[:, 0, :], pat[:, 0, :], mask_sb[:, 0, :]
                    )
                    nc.vector.tensor_mul(
                        at_m[:, 1, P:], pat[:, 1, P:], mask_sb[:, 1, P:]
                    )

            # ---- output (natural layout): out[t, j] = at^T v + eq^T S ----
            oc = spool.tile([P, 2, J], F32, tag="oc")
            pot = potp.tile([P, 2, J], F32, tag="po")
            rq_b = rq4[:, :, :, :, None].broadcast_to([P, 2, 2, 2, DH])
            for tt in range(2):
                for jt in range(2):
                    for half in range(2):
                        h = 2 * jt + half
                        rows = slice(64 * half, 64 * half + 64)
                        jcols = slice(h * DH, (h + 1) * DH)
                        at_m = at_mt[(c % 2, jt, half)]
                        nc.tensor.matmul(
                            pot[:, tt, jcols],
                            at_m[:, 0, tt * P:(tt + 1) * P],
                            v_sb[:, 0, jcols],
                            start=True,
                            stop=False,
                        )
                        nc.tensor.matmul(
                            pot[:, tt, jcols],
                            at_m[:, 1, tt * P:(tt + 1) * P],
                            v_sb[:, 1, jcols],
                            start=False,
                            stop=(c == 0),
                        )
                        if c > 0:
                            nc.tensor.matmul(
                                pot[:, tt, jcols],
                                eq_e[rows, jt, tt * P:(tt + 1) * P],
                                S16[rows, jt, :],
                                start=False,
                                stop=True,
                            )
                # per-t-tile multiply+store: the first store starts while
                # the second tile's output matmuls run (ACT HWDGE ring)
                nc.vector.tensor_mul(
                    oc[:, tt, :].rearrange("p (jt h2 d) -> p jt h2 d",
                                           jt=2, h2=2),
                    pot[:, tt, :].rearrange("p (jt h2 d) -> p jt h2 d",
                                            jt=2, h2=2),
                    rq_b[:, tt],
                )
                nc.scalar.dma_start(out_r[:, c, tt], oc[:, tt, :])

            # ---- state update: S += ek_nat^T v (diagonal head blocks) ----
            if not last:
                for jt in range(2):
                    # full-width rhs keeps a single matmul per s-tile; the
                    # off-pair half of the output is unused.
                    pds = patp.tile([P, J], F32, tag="pa")
                    for st in range(2):
                        nc.tensor.matmul(
                            pds[:],
                            ekn_sb[:, st, jt * P:(jt + 1) * P],
                            v_sb[:, st, :],
                            start=(st == 0),
                            stop=(st == 1),
                        )
                    for half in range(2):
                        rows = slice(64 * half, 64 * half + 64)
                        col = jt * P + 64 * half
                        if c == 0:
                            nc.vector.tensor_copy(
                                S_sb[rows, jt, :], pds[rows, col:col + 64]
                            )
                        else:
                            nc.vector.tensor_add(
                                S_sb[rows, jt, :],
                                S_sb[rows, jt, :],
                                pds[rows, col:col + 64],
                            )
                # bf16 shadow for the next chunk's apply matmuls, on the
                # otherwise-idle gpsimd engine (no ACT/DVE head-of-line)
                nc.gpsimd.tensor_copy(S16[:], S_sb[:])

    nc.finalize()
    return nc


def _host_inputs(query, key, Wq, Wk, Wv):
    """Build the 8 per-core input maps (host-side layout prep)."""
    import ml_dtypes
    FP8NP = ml_dtypes.float8_e4m3

    def split_fp8(a):
        hi = a.astype(FP8NP)
        lo = (a - hi.astype(np.float32)).astype(FP8NP)
        return hi, lo

    def x_layout(xn):
        # xn [L, D] fp32 -> hi/lo [P, NCH, 2, KB, 2, C] fp8
        xt = np.ascontiguousarray(xn.T.astype(np.float32))  # [D, L]
        hi, lo = split_fp8(xt)
        out = np.empty((P, NCH, 2, KB, 2, C), dtype=FP8NP)
        for i, a in enumerate((hi, lo)):
            # d = kb*256 + pl*128 + p ; t = c*256 + tt
            r = a.reshape(KB, 2, P, NCH, C)          # [kb, pl, p, c, t]
            out[:, :, i] = r.transpose(2, 3, 0, 1, 4)  # [p, c, kb, pl, t]
        return np.ascontiguousarray(out)

    def w_layout(Ws, cols):
        # -> [P, 3, 2, KB, 2, J] fp8, scaled by WSCALE
        out = np.empty((P, 3, 2, KB, 2, J), dtype=FP8NP)
        for pi, W in enumerate(Ws):
            wt = np.ascontiguousarray(W[cols, :].T.astype(np.float32)) * WSCALE
            hi, lo = split_fp8(wt)                   # [D, J]
            for i, a in enumerate((hi, lo)):
                r = a.reshape(KB, 2, P, J)           # [kb, pl, p, j]
                out[:, pi, i] = r.transpose(2, 0, 1, 3)
        return np.ascontiguousarray(out)

    mask = np.zeros((P, 2, C), np.float32)
    for st in range(2):
        s = st * P + np.arange(P)[:, None]
        t = np.arange(C)[None, :]
        mask[:, st, :] = (s <= t).astype(np.float32)
    eones1 = np.zeros((P, 2), np.float32)
    eones1[:64, 0] = 1.0
    eones1[64:, 1] = 1.0
    eones64 = (eones1 * WSCALE)
    ident = np.eye(P, dtype=np.float32)

    bf = ml_dtypes.bfloat16
    per_batch = {n: (x_layout(query[n]), x_layout(key[n])) for n in range(2)}

    in_maps = []
    for core in range(N_CORES):
        n, g = core // 4, core % 4
        xq_a, xk_a = per_batch[n]
        cols = slice(g * J, (g + 1) * J)
        in_maps.append({
            "xq": xq_a,
            "xk": xk_a,
            "wa": w_layout((Wq, Wk, Wv), cols),
            "mask": mask,
            "eones1": eones1.astype(bf),
            "eones64": eones64.astype(bf),
            "ident": ident.astype(bf),
        })
    return in_maps


_NC_LOCK = threading.Lock()
_NC_CACHE = {}


def _get_nc():
    with _NC_LOCK:
        if "nc" not in _NC_CACHE:
            _NC_CACHE["nc"] = _build_nc()
        return _NC_CACHE["nc"]


def kernel(query, key, Wq, Wk, Wv, _trace=False, _trace_kwargs=None):
    query = np.asarray(query)
    key = np.asarray(key)
    Wq = np.asarray(Wq)
    Wk = np.asarray(Wk)
    Wv = np.asarray(Wv)

    nc = _get_nc()
    in_maps = _host_inputs(query, key, Wq, Wk, Wv)
    res = run_bass_kernel_spmd(
        nc, in_maps, core_ids=list(range(N_CORES)),
        trace=_trace, **(_trace_kwargs or {}),
    )

    out = np.empty((2, L, D), np.float32)
    for core, r in enumerate(res.results):
        n, g = core // 4, core % 4
        out[n, :, g * J:(g + 1) * J] = r["out_nat"]
    if _trace:
        kernel.last_results = res
    return out
